# revision 8
# baseline (speedup 1.0000x reference)
"""Trainium2 Bass kernel for HeightCompressionMean (segment_reduce).

Computes: scatter 400k voxel feature rows [N,128] into a dense grid
[B=4, C=128, D=2, H=256, W=256] (duplicate (b,z,y,x) cells resolve
last-write-wins, matching jax .at[].set on CPU), then mean over depth
-> [B, C, H, W].

Strategy (8 cores, no cross-core communication):
 - Shard by H: core k owns output rows y in [32k, 32k+32).
 - Host: per core, resolve the winning voxel per dense cell
   (index-only work), pack winner feature rows in cell order, and
   build int16 gather-index lists (one per (batch, half-plane),
   covering both z-planes).
 - Device: dma_gather (SWDGE ucode) pulls 512B feature rows from HBM
   at full DMA bandwidth, landing row j on partition j%128 — i.e.
   [cell, C] tiles ready for PE transpose. DVE adds the two z-planes,
   PE transposes 128x128 tiles to channel-major, the Activation engine
   applies the 1/D scale during the PSUM->SBUF copy, and one DMA per
   half-plane writes the output.
"""

import sys

for _p in ("/opt/trn_rl_repo", "/root/.axon_site/_ro/trn_rl_repo"):
    if _p not in sys.path:
        sys.path.append(_p)

import numpy as np

from concourse import bass, mybir, bacc
import concourse.tile as tile
from concourse.bass_utils import run_bass_kernel_spmd
from concourse.library_config import mlp
from concourse.masks import make_identity

M = 8          # cores
B = 4          # batch
D = 2          # depth (z)
H = 256
W = 256
C = 128        # channels
H_LOC = H // M           # 32 rows of y per core
NPLANE = H_LOC * W       # 8192 cells per (b, z) plane per core
NHALF = NPLANE // 2      # 4096 cells per half-plane
NCELL = B * D * NPLANE   # 65536 dense cells per core
PHROWS = 32768           # feature rows per phase (int16 index space)
ZROW = PHROWS - 1        # zero row per phase (free unless phase is full,
                         # in which case there are no empty cells)
NGATHER = B * 2          # one gather per (b, half), 2*NHALF rows each
GIDX = 2 * NHALF         # indices per gather (z0 block then z1 block)

_nc_cache = {}


def _build(repeat=1):
    if repeat in _nc_cache:
        return _nc_cache[repeat]

    f32 = mybir.dt.float32
    i16 = mybir.dt.int16

    nc = bacc.Bacc("TRN2", target_bir_lowering=False, debug=False)
    feats = nc.dram_tensor("feats", [2 * PHROWS, C], f32, kind="ExternalInput")
    widx = nc.dram_tensor("widx", [128, NGATHER * GIDX // 16], i16, kind="ExternalInput")
    outp = nc.dram_tensor("out", [B, C, NPLANE], f32, kind="ExternalOutput")

    with tile.TileContext(nc) as tc:
        with (
            tc.tile_pool(name="const", bufs=1) as constp,
            tc.tile_pool(name="gpool", bufs=2) as gpool,
            tc.tile_pool(name="spool", bufs=2) as spool,
            tc.tile_pool(name="opool", bufs=2) as opool,
            tc.tile_pool(name="psum", bufs=4, space="PSUM") as psump,
        ):
            nc.gpsimd.load_library(mlp)
            idx_sb = constp.tile([128, NGATHER * GIDX // 16], i16)
            nc.sync.dma_start(out=idx_sb[:], in_=widx[:])
            ident = constp.tile([128, 128], f32)
            make_identity(nc, ident[:])

            for _ in range(repeat):
                for b in range(B):
                    for half in range(2):
                        src = feats[(b // 2) * PHROWS : (b // 2 + 1) * PHROWS, :]
                        g = gpool.tile([128, 2 * NHALF // 128, C], f32)
                        for z in range(D):
                            for i in range(NHALF // 1024):
                                q = (((b * D + z) * 2 + half) * (NHALF // 1024)) + i
                                sl = (z * (NHALF // 1024) + i) * 8
                                nc.gpsimd.dma_gather(
                                    g[:, sl : sl + 8, :],
                                    src,
                                    idx_sb[:, q * 64 : (q + 1) * 64],
                                    1024,
                                    1024,
                                    C,
                                )
                        s = spool.tile([128, NHALF], f32)
                        nc.vector.tensor_tensor(
                            out=s[:].rearrange("p (a b) -> p a b", b=C),
                            in0=g[:, : NHALF // 128, :],
                            in1=g[:, NHALF // 128 :, :],
                            op=mybir.AluOpType.add,
                        )
                        o = opool.tile([128, NHALF], f32)
                        for grp in range(NHALF // 512):
                            pt = psump.tile([128, 512], f32, space="PSUM")
                            for j in range(4):
                                t = grp * 4 + j
                                nc.tensor.transpose(
                                    out=pt[:, j * 128 : (j + 1) * 128],
                                    in_=s[:, t * 128 : (t + 1) * 128],
                                    identity=ident[:],
                                )
                            nc.scalar.activation(
                                out=o[:, grp * 512 : (grp + 1) * 512],
                                in_=pt[:],
                                func=mybir.ActivationFunctionType.Copy,
                                scale=0.5,
                            )
                        nc.sync.dma_start(
                            out=outp[b, :, half * NHALF : (half + 1) * NHALF],
                            in_=o[:],
                        )

    nc.compile()
    _nc_cache[repeat] = nc
    return nc


def _shard_inputs(features, b_idx, z_idx, y_idx, x_idx):
    """Per-core host prep: winner-per-cell resolution + packing."""
    features = np.ascontiguousarray(features, dtype=np.float32)
    b_idx = np.asarray(b_idx).astype(np.int64)
    z_idx = np.asarray(z_idx).astype(np.int64)
    y_idx = np.asarray(y_idx).astype(np.int64)
    x_idx = np.asarray(x_idx).astype(np.int64)

    slab = y_idx // H_LOC
    in_maps = []
    for k in range(M):
        fi = np.flatnonzero(slab == k)
        cell = (
            (b_idx[fi] * D + z_idx[fi]) * H_LOC + (y_idx[fi] - k * H_LOC)
        ) * W + x_idx[fi]

        win = np.full(NCELL, -1, dtype=np.int64)
        win[cell] = np.arange(fi.size)  # duplicate cells: last write wins

        feats_k = np.zeros((2 * PHROWS, C), dtype=np.float32)
        wp = np.empty(NCELL, dtype=np.int16)
        for ph in range(2):
            wloc = win[ph * PHROWS : (ph + 1) * PHROWS]
            occ = wloc >= 0
            nsurv = int(occ.sum())
            assert nsurv <= PHROWS
            if nsurv:
                feats_k[ph * PHROWS : ph * PHROWS + nsurv] = (
                    features[fi[wloc[occ]]]
                )
            wp_ph = np.full(PHROWS, ZROW, dtype=np.int16)
            wp_ph[occ] = np.arange(nsurv, dtype=np.int16)
            wp[ph * PHROWS : (ph + 1) * PHROWS] = wp_ph

        # index lists: one per (b, z, half) = wp in natural cell order;
        # wrap each list over 16 partitions, replicate x8.
        L = wp.reshape(64, 1024)
        wrap = L.reshape(64, 64, 16)  # [g, s, p%16]
        dev16 = np.ascontiguousarray(
            wrap.transpose(2, 0, 1).reshape(16, -1)
        )  # [p%16, NGATHER*GIDX//16]
        wp_dev = np.tile(dev16, (8, 1))  # replicate over 128 partitions
        in_maps.append({"feats": feats_k, "widx": wp_dev})
    return in_maps


def kernel(features, b_idx, z_idx, y_idx, x_idx, batch_size, depth, height, width):
    assert int(batch_size) == B and int(depth) == D
    assert int(height) == H and int(width) == W

    nc = _build()
    in_maps = _shard_inputs(features, b_idx, z_idx, y_idx, x_idx)
    res = run_bass_kernel_spmd(nc, in_maps, list(range(M)))

    out = np.empty((B, C, H, W), dtype=np.float32)
    for k in range(M):
        out[:, :, k * H_LOC : (k + 1) * H_LOC, :] = (
            res.results[k]["out"].reshape(B, C, H_LOC, W)
        )
    return out


if __name__ == "__main__":
    rng = np.random.default_rng(0)
    N = 400000
    inputs = dict(
        features=rng.standard_normal((N, C), dtype=np.float32),
        b_idx=rng.integers(0, B, N).astype(np.int32),
        z_idx=rng.integers(0, D, N).astype(np.int32),
        y_idx=rng.integers(0, H, N).astype(np.int32),
        x_idx=rng.integers(0, W, N).astype(np.int32),
        batch_size=B, depth=D, height=H, width=W,
    )
    out = kernel(**inputs)
    print(out.shape, out.dtype)
